# revision 1
# baseline (speedup 1.0000x reference)
"""Causal attention block (B=4, S=2048, D=1024, H=16) on 8 Trainium2 NeuronCores.

Sharding: core c = (batch b = c//2, head-group hg = c%2 of 8 heads).
Each core computes QKV projection for its batch restricted to its heads'
columns, causal flash-style attention for its 8 heads, and a partial output
projection (its heads' rows of W_proj). Host sums the two partial outputs
per batch pair and returns the full [4, 2048, 1024] result.

Layout choices (per core):
  - x arrives pre-transposed as xT [1024, 2048] so the embedding dim (the
    matmul contraction dim) is the SBUF partition dim.
  - q, k are produced transposed: qT/kT [512 cols, 2048 tokens] stored as
    [128, 4, 2048] tiles; head h lives in tile chunk h//2, partitions
    (h%2)*64..+64. 1/sqrt(hd) folded into W_q on the host.
  - v is produced in natural [token, col] orientation as [128, 16, 8, 65]
    (key-block, head, 64 v-cols + a ones column for softmax denominators).
  - scores are computed transposed, sT[k, q] = kT_block.T @ qT, exp'd with no
    max subtraction (scores are ~N(0,1); fp32 exp cannot overflow), causal
    diagonal masked with affine_select, then att@v accumulates over key
    blocks in PSUM; the ones column yields the denominator row.
  - normalization: reciprocal of the denominator row, partition_broadcast,
    multiply, staged to the out-projection lhsT layout via SBUF->SBUF DMA.
"""

import numpy as np

import concourse.bass as bass
import concourse.mybir as mybir
import concourse.tile as tile
from concourse import bacc
from concourse.bass_utils import run_bass_kernel_spmd
from concourse.masks import make_upper_triangular

F32 = mybir.dt.float32
F32R = mybir.dt.float32r
EMB = 1024
HEADS = 16
HD = 64
B = 4
S = 2048
NCORES = 8
HPC = 8           # heads per core
CD = HPC * HD     # 512 cols per core for each of q/k/v
NKB = S // 128    # 16 key blocks
NQC = S // 512    # 4 query chunks

_EXP = mybir.ActivationFunctionType.Exp


def _build_module(debug_dump=False):
    nc = bacc.Bacc("TRN2", target_bir_lowering=False, debug=False)
    xT = nc.declare_dram_parameter("xT", [EMB, S], F32R, isOutput=False)
    wq = nc.declare_dram_parameter("wq", [EMB, CD], F32R, isOutput=False)
    wk = nc.declare_dram_parameter("wk", [EMB, CD], F32R, isOutput=False)
    wv = nc.declare_dram_parameter("wv", [EMB, CD], F32R, isOutput=False)
    wp = nc.declare_dram_parameter("wp", [CD, EMB], F32R, isOutput=False)
    bias = nc.declare_dram_parameter("bias", [1, EMB], F32, isOutput=False)
    ones = nc.declare_dram_parameter("ones", [NKB, HPC], F32R, isOutput=False)
    y = nc.declare_dram_parameter("y", [S, EMB], F32, isOutput=True)
    dbg = None
    if debug_dump:
        dbg = {
            "qt": nc.declare_dram_parameter("dbg_qt", [128, 4, S], F32, isOutput=True),
            "kt": nc.declare_dram_parameter("dbg_kt", [128, 4, S], F32, isOutput=True),
            "vx": nc.declare_dram_parameter(
                "dbg_vx", [128, NKB, HPC, HD + 1], F32, isOutput=True
            ),
            "oT": nc.declare_dram_parameter(
                "dbg_oT", [NQC, 128, 4, 512], F32, isOutput=True
            ),
            "den": nc.declare_dram_parameter(
                "dbg_den", [NQC, HPC, 1, 512], F32, isOutput=True
            ),
            "bc": nc.declare_dram_parameter(
                "dbg_bc", [NQC, HPC, 64, 512], F32, isOutput=True
            ),
            "un": nc.declare_dram_parameter(
                "dbg_un", [NQC, HPC, HD, 512], F32, isOutput=True
            ),
        }

    with tile.TileContext(nc) as tc:
        _body(tc, nc, xT, wq, wk, wv, wp, bias, ones, y, dbg)
    nc.compile()
    return nc


def _body(tc, nc, xT, wq, wk, wv, wp, bias, ones, y, dbg=None):
    from contextlib import ExitStack

    with ExitStack() as ctx:
        persist = ctx.enter_context(tc.tile_pool(name="persist", bufs=1))
        qt = persist.tile([128, 4, S], F32R, tag="qt")
        kt = persist.tile([128, 4, S], F32R, tag="kt")
        vx = persist.tile([128, NKB, HPC, HD + 1], F32R, tag="vx")

        # ones column for denominators (DMA-broadcast from host input; memset
        # cannot produce float32r)
        nc.sync.dma_start(
            out=vx[:, :, :, HD : HD + 1], in_=ones[:].partition_broadcast(128)
        )
        # causal mask for diagonal blocks: tri[p, f] = 1.0 iff f >= p
        tri = persist.tile([128, 128], F32, tag="tri")
        make_upper_triangular(nc, tri[:], val=1.0, diag=True)
        tri2 = persist.tile([128, 256], F32, tag="tri2")
        nc.gpsimd.memset(tri2[:, 0:128], 0.0)
        make_upper_triangular(nc, tri2[:, 128:256], val=1.0, diag=True)

        # ---------------- Phase 1: QKV projections ----------------
        with ExitStack() as p1:
            xt_pool = p1.enter_context(tc.tile_pool(name="xt", bufs=2))
            w_pool = p1.enter_context(tc.tile_pool(name="w", bufs=4))
            wv_pool = p1.enter_context(tc.tile_pool(name="wvp", bufs=1))
            qkv_ps = p1.enter_context(
                tc.tile_pool(name="qkvps", bufs=4, space="PSUM")
            )

            wv_sb = wv_pool.tile([128, 8, CD], F32R, tag="wv")

            for half in range(2):
                t0 = half * 1024
                xt_sb = xt_pool.tile([128, 8, 1024], F32R, tag="xt")
                # load the n=0 token halves of every chunk first: the first
                # matmul group needs only those, halving the startup DMA gate
                for n2 in range(2):
                    for kc in range(8):
                        c0 = t0 + n2 * 512
                        nc.sync.dma_start(
                            out=xt_sb[:, kc, n2 * 512 : (n2 + 1) * 512],
                            in_=xT[kc * 128 : (kc + 1) * 128, c0 : c0 + 512],
                        )
                # qT and kT (transposed outputs)
                for qk, wdram, dst in ((0, wq, qt), (1, wk, kt)):
                    for m in range(4):
                        wt = w_pool.tile([128, 8, 128], F32R, tag="w")
                        nc.scalar.dma_start(
                            out=wt[:],
                            in_=wdram[:, m * 128 : (m + 1) * 128].rearrange(
                                "(c p) m -> p c m", p=128
                            ),
                        )
                        for n in range(2):
                            ps = qkv_ps.tile([128, 512], F32, tag="qkvps")
                            for kc in range(8):
                                nc.tensor.matmul(
                                    ps[:],
                                    lhsT=(wt[:, kc, :]),
                                    rhs=(xt_sb[:, kc, n * 512 : (n + 1) * 512]),
                                    start=(kc == 0),
                                    stop=(kc == 7),
                                )
                            col = t0 + n * 512
                            nc.vector.tensor_copy(
                                out=dst[:, m, col : col + 512], in_=ps[:]
                            )
                # v (natural orientation, strided into vx)
                if half == 0:
                    for kc in range(8):
                        nc.gpsimd.dma_start(
                            out=wv_sb[:, kc, :],
                            in_=wv[kc * 128 : (kc + 1) * 128, :],
                        )
                for tc8 in range(8):
                    tg = half * 8 + tc8
                    ps = qkv_ps.tile([128, 512], F32, tag="qkvps")
                    for kc in range(8):
                        nc.tensor.matmul(
                            ps[:],
                            lhsT=(xt_sb[:, kc, tc8 * 128 : (tc8 + 1) * 128]),
                            rhs=(wv_sb[:, kc, :]),
                            start=(kc == 0),
                            stop=(kc == 7),
                        )
                    nc.vector.tensor_copy(
                        out=vx[:, tg, :, 0:HD],
                        in_=ps[:].rearrange("p (h d) -> p h d", h=HPC),
                    )

        if dbg is not None:
            nc.sync.dma_start(out=dbg["qt"][:], in_=qt[:])
            nc.sync.dma_start(out=dbg["kt"][:], in_=kt[:])
            nc.sync.dma_start(out=dbg["vx"][:], in_=vx[:])

        # ---------------- Phase 2+3: attention + output projection ----------------
        with ExitStack() as p2:
            misc = p2.enter_context(tc.tile_pool(name="misc", bufs=1))
            s_pool = p2.enter_context(tc.tile_pool(name="sps", bufs=3, space="PSUM"))
            outT_pool = p2.enter_context(
                tc.tile_pool(name="outTps", bufs=3, space="PSUM")
            )
            y_pool = p2.enter_context(tc.tile_pool(name="yps", bufs=2, space="PSUM"))
            e_pool = p2.enter_context(tc.tile_pool(name="es", bufs=5))
            r_pool = p2.enter_context(tc.tile_pool(name="recip", bufs=3))
            b_pool = p2.enter_context(tc.tile_pool(name="bcast", bufs=3))
            st_pool = p2.enter_context(tc.tile_pool(name="stage", bufs=4))
            oT_pool = p2.enter_context(tc.tile_pool(name="oT", bufs=2))
            ysb_pool = p2.enter_context(tc.tile_pool(name="ysb", bufs=3))

            scr_pool = p2.enter_context(
                tc.tile_pool(name="scr", bufs=6, space="DRAM")
            )
            wp_sb = misc.tile([128, 4, EMB], F32R, tag="wp")
            nc.sync.dma_start(
                out=wp_sb[:], in_=wp[:].rearrange("(c p) e -> p c e", p=128)
            )
            bias_sb = misc.tile([128, 1, EMB], F32, tag="bias")
            nc.sync.dma_start(out=bias_sb[:], in_=bias[:].partition_broadcast(128))
            # ones row at partition 64 (same base partition as the denominator
            # row) for the PE-broadcast used on the final head
            onescol = misc.tile([65, 64], F32, tag="onescol")
            nc.sync.dma_start(
                out=onescol[64:65, 0:64], in_=ones[0:8, 0:8].bitcast(F32)
            )

            def make_y_group(oT_prev, qc_prev, tc4, ncol):
                def emit():
                    row = qc_prev * 512 + tc4 * 128
                    y_ps = y_pool.tile([128, 512], F32, tag="y")
                    for kc in range(4):
                        nc.tensor.matmul(
                            y_ps[:],
                            lhsT=(oT_prev[:, kc, tc4 * 128 : (tc4 + 1) * 128]),
                            rhs=(wp_sb[:, kc, ncol * 512 : (ncol + 1) * 512]),
                            start=(kc == 0),
                            stop=(kc == 3),
                        )
                    y_sb = ysb_pool.tile([128, 512], F32, tag="ysb")
                    nc.vector.tensor_add(
                        y_sb[:],
                        y_ps[:],
                        bias_sb[:, 0, ncol * 512 : (ncol + 1) * 512],
                    )
                    nc.sync.dma_start(
                        out=y[row : row + 128, ncol * 512 : (ncol + 1) * 512],
                        in_=y_sb[:],
                    )

                return emit

            deferred_y = []
            for qc in range(NQC):
                oT = oT_pool.tile([128, 4, 512], F32R, tag="oT")
                kb_max = 4 * qc + 4
                head_order = (
                    (1, 3, 5, 7, 0, 2, 4, 6) if qc == NQC - 1 else tuple(range(HPC))
                )
                for h in head_order:
                    m, po = h // 2, (h % 2) * 64
                    outT_ps = outT_pool.tile([HD + 1, 512], F32, tag="outT")
                    # previous chunk's output projection fills the PE while
                    # this head's exp chain runs on the scalar engine
                    if deferred_y:
                        deferred_y.pop(0)()

                    def emit_av(pending_infos, pending_es):
                        for j, kb, q0, nq, diag in pending_infos:
                            nc.tensor.matmul(
                                out=outT_ps[:, q0:512],
                                lhsT=(vx[:, kb, h, :]),
                                rhs=(pending_es[:, 0:nq]),
                                start=(kb == 0),
                                stop=(kb == kb_max - 1),
                            )

                    pending = None
                    for kb in range(kb_max):
                        r = kb * 128 - qc * 512
                        q0 = max(r, 0)
                        nq = 512 - q0
                        pad = r >= 0 and nq < 256
                        if pad:
                            # keep the moving dim >= 256 (fp32r runs 4x
                            # slower below that); mask the extra columns
                            q0, nq = 256, 256
                        s_ps = s_pool.tile([128, 512], F32, tag="s")
                        es = e_pool.tile([128, 512], F32R, tag="es")
                        nc.tensor.matmul(
                            out=s_ps[:, 0:nq],
                            lhsT=(kt[po : po + 64, m, kb * 128 : (kb + 1) * 128]),
                            rhs=(qt[po : po + 64, m, qc * 512 + q0 : (qc + 1) * 512]),
                            start=True,
                            stop=True,
                        )
                        nc.scalar.activation(
                            out=es[:, 0:nq], in_=s_ps[:, 0:nq], func=_EXP
                        )
                        if pad:
                            nc.vector.tensor_mul(es[:, 0:256], es[:, 0:256], tri2[:])
                        elif r >= 0:
                            nc.vector.tensor_mul(es[:, 0:128], es[:, 0:128], tri[:])
                        # av matmuls run one block behind so the PE never
                        # waits on the exp of the block it just produced
                        if pending is not None:
                            emit_av(*pending)
                        pending = ([(0, kb, q0, nq, r >= 0)], es)
                    if pending is not None:
                        emit_av(*pending)
                    # normalize: divide by denominator row (row HD)
                    recip = r_pool.tile([HD + 1, 512], F32, tag="recip")
                    nc.vector.reciprocal(
                        recip[HD : HD + 1, :], outT_ps[HD : HD + 1, :]
                    )
                    bcast = b_pool.tile([64, 512], F32, tag="bcast")
                    if qc == NQC - 1 and h == head_order[-1]:
                        # final head: its normalize chain is fully exposed at
                        # the kernel tail, so broadcast via an idle-PE matmul
                        # (ones column x reciprocal row) instead of the
                        # higher-latency DRAM-bounce DMA pair
                        bc_ps = s_pool.tile([64, 512], F32, tag="s")
                        nc.tensor.matmul(
                            out=bc_ps[:],
                            lhsT=onescol[64:65, :],
                            rhs=recip[HD : HD + 1, :],
                            start=True,
                            stop=True,
                        )
                        nc.vector.tensor_copy(out=bcast[:], in_=bc_ps[:])
                    else:
                        scr = scr_pool.tile([1, 512], F32, tag="scr")
                        nc.sync.dma_start(out=scr[:], in_=recip[HD : HD + 1, :])
                        nc.sync.dma_start(
                            out=bcast[:], in_=scr[0:1, :].partition_broadcast(64)
                        )
                    if po == 0:
                        nc.vector.tensor_mul(
                            oT[0:HD, m, :], outT_ps[0:HD, :], bcast[:]
                        )
                    else:
                        stage = st_pool.tile([64, 512], F32R, tag="stage")
                        nc.vector.tensor_mul(stage[:], outT_ps[0:HD, :], bcast[:])
                        nc.sync.dma_start(out=oT[po : po + 64, m, :], in_=stage[:])
                    if dbg is not None:
                        den_sb = st_pool.tile([HD + 1, 512], F32, tag="dbgden")
                        nc.vector.tensor_copy(
                            out=den_sb[HD : HD + 1, :], in_=outT_ps[HD : HD + 1, :]
                        )
                        nc.sync.dma_start(
                            out=dbg["den"][qc, h], in_=den_sb[HD : HD + 1, :]
                        )
                        un_sb = st_pool.tile([HD, 512], F32, tag="dbgun")
                        nc.vector.tensor_copy(out=un_sb[:], in_=outT_ps[0:HD, :])
                        nc.sync.dma_start(out=dbg["un"][qc, h], in_=un_sb[:])
                        nc.sync.dma_start(out=dbg["bc"][qc, h], in_=bcast[:])

                if dbg is not None:
                    nc.sync.dma_start(out=dbg["oT"][qc], in_=oT[:])

                # defer this chunk's output projection into the next chunk's
                # head loop (emitted one group per head)
                assert not deferred_y
                deferred_y = [
                    make_y_group(oT, qc, tc4, ncol)
                    for tc4 in range(4)
                    for ncol in range(2)
                ]
            for emit in deferred_y:
                emit()


_MODULE = None


def _get_module():
    global _MODULE
    if _MODULE is None:
        _MODULE = _build_module()
    return _MODULE


def _make_in_maps(x, W_qkv, W_proj, b_proj):
    scale = np.float32(1.0 / np.sqrt(HD))
    bias_half = (np.asarray(b_proj, dtype=np.float32) * 0.5).reshape(1, EMB)
    in_maps = []
    for c in range(NCORES):
        b, hg = c // 2, c % 2
        cols = slice(hg * CD, (hg + 1) * CD)
        in_maps.append(
            {
                "xT": np.ascontiguousarray(np.asarray(x[b], dtype=np.float32).T),
                "wq": np.ascontiguousarray(W_qkv[:, 0:EMB][:, cols]) * scale,
                "wk": np.ascontiguousarray(W_qkv[:, EMB : 2 * EMB][:, cols]),
                "wv": np.ascontiguousarray(W_qkv[:, 2 * EMB : 3 * EMB][:, cols]),
                "wp": np.ascontiguousarray(W_proj[cols, :]),
                "bias": bias_half,
                "ones": np.ones((NKB, HPC), dtype=np.float32),
            }
        )
    return in_maps


def kernel(x, W_qkv, W_proj, b_proj, _trace=False, _trace_kwargs=None):
    x = np.asarray(x, dtype=np.float32)
    W_qkv = np.asarray(W_qkv, dtype=np.float32)
    W_proj = np.asarray(W_proj, dtype=np.float32)
    b_proj = np.asarray(b_proj, dtype=np.float32)

    nc = _get_module()
    in_maps = _make_in_maps(x, W_qkv, W_proj, b_proj)
    res = run_bass_kernel_spmd(
        nc, in_maps, list(range(NCORES)), trace=_trace, **(_trace_kwargs or {})
    )
    out = np.empty((B, S, EMB), dtype=np.float32)
    for b in range(B):
        out[b] = res.results[2 * b]["y"] + res.results[2 * b + 1]["y"]
    if _trace:
        return out, res
    return out



# revision 7
# speedup vs baseline: 1.0952x; 1.0952x over previous
"""Causal attention block (B=4, S=2048, D=1024, H=16) on 8 Trainium2 NeuronCores.

Sharding: core c = (batch b = c//2, head-group hg = c%2 of 8 heads).
Each core computes QKV projection for its batch restricted to its heads'
columns, causal attention for its 8 heads, and a partial output projection
(its heads' rows of W_proj). Host sums the two partial outputs per batch
pair and returns the full [4, 2048, 1024] result.

v2 design (cost model: matmul = moving-rows only; DVE/Act = free-size only):
  - Everything bf16 (matmul speed identical to fp32r>=256, halves DMA/SBUF,
    removes the fp32r <256-moving-dim penalty so diagonal blocks need no pad).
  - Scores computed transposed sT[k, q] per 2-key-block group into a PSUM
    group tile [128, 2, 512]; ONE exp instruction per full group (Act cost
    is per-free-element, so fewer/larger activations cut the fixed 185ns
    per-instruction overhead); staircase (diagonal) blocks get exact-region
    exps. exp with no max subtraction (scores ~N(0,1), fp32 exp safe).
  - AV restructured: es is the STATIONARY operand (lhsT [128 keys, 128 q])
    and v streams (65 rows incl. a ones column for the denominator), giving
    o[q, v] naturally oriented at ~2x fewer PE rows than streaming scores.
  - Normalization in natural orientation: one batched DVE reciprocal per
    (head, qc) of the 4 denominator columns, then per-qj DVE tensor_scalar
    multiplies (per-partition scalar broadcast; no DRAM-bounce broadcast).
  - Head pairs share an o_norm [128 q, 128] tile (even head cols 0:64, odd
    64:128); one DMA-engine transpose (bf16 xbar, 14ns/tile) per qj lands
    both heads directly into the output-projection lhsT layout. No PE
    transposes, no PSUM->SBUF staging copies.
  - Output projection y = oT.T @ wp in bf16, bias added on DVE, y stored
    bf16 (host upcasts and sums the two partial cores).
  - Software pipelining: attention for qc=0,1 (which only needs tokens
    0:1024 of q/k/v) is interleaved with phase-1 half-1 QKV projection;
    AV/normalize/y work is deferred into later heads' QK/exp slots via a
    budgeted filler queue so the PE never idles while Act runs exps.
"""

from collections import deque

import numpy as np

import concourse.bass as bass
import concourse.mybir as mybir
import concourse.tile as tile
from concourse import bacc
from concourse.bass_utils import run_bass_kernel_spmd
from concourse.masks import make_upper_triangular

F32 = mybir.dt.float32
BF16 = mybir.dt.bfloat16
EMB = 1024
HEADS = 16
HD = 64
B = 4
S = 2048
NCORES = 8
HPC = 8           # heads per core
CD = HPC * HD     # 512 cols per core for each of q/k/v
NKB = S // 128    # 16 key blocks
NQC = S // 512    # 4 query chunks

_EXP = mybir.ActivationFunctionType.Exp


def _build_module():
    nc = bacc.Bacc("TRN2", target_bir_lowering=False, debug=False)
    xT = nc.declare_dram_parameter("xT", [EMB, S], BF16, isOutput=False)
    wq = nc.declare_dram_parameter("wq", [EMB, CD], BF16, isOutput=False)
    wk = nc.declare_dram_parameter("wk", [EMB, CD], BF16, isOutput=False)
    wv = nc.declare_dram_parameter("wv", [EMB, CD], BF16, isOutput=False)
    wp = nc.declare_dram_parameter("wp", [CD, EMB], BF16, isOutput=False)
    bias = nc.declare_dram_parameter("bias", [1, EMB], F32, isOutput=False)
    y = nc.declare_dram_parameter("y", [S, EMB], BF16, isOutput=True)

    with tile.TileContext(nc) as tc:
        _body(tc, nc, xT, wq, wk, wv, wp, bias, y)
    nc.compile()
    return nc


def _body(tc, nc, xT, wq, wk, wv, wp, bias, y):
    from contextlib import ExitStack

    with ExitStack() as ctx:
        persist = ctx.enter_context(tc.tile_pool(name="persist", bufs=1))
        qt = persist.tile([128, 4, S], BF16, tag="qt")
        kt = persist.tile([128, 4, S], BF16, tag="kt")
        vx = persist.tile([128, NKB, HPC, HD + 1], BF16, tag="vx")

        # ones column for denominators (bf16 memset works; f32r did not)
        nc.gpsimd.memset(vx[:, :, :, HD : HD + 1], 1.0)
        # causal mask for diagonal blocks: tri[p, f] = 1.0 iff f >= p
        tri = persist.tile([128, 128], BF16, tag="tri")
        make_upper_triangular(nc, tri[:], val=1.0, diag=True)

        wp_sb = persist.tile([128, 4, EMB], BF16, tag="wp")
        bias_sb = persist.tile([128, 1, EMB], F32, tag="bias")

        # ---------------- pools ----------------
        # PSUM budget (8 banks): qkv 2 + s 2x2 + o 1 + y 1 = 8
        qkv_ps = ctx.enter_context(tc.tile_pool(name="qkvps", bufs=2, space="PSUM"))
        s_pool = ctx.enter_context(tc.tile_pool(name="sps", bufs=2, space="PSUM"))
        o_pool = ctx.enter_context(tc.tile_pool(name="ops", bufs=1, space="PSUM"))
        y_pool = ctx.enter_context(tc.tile_pool(name="yps", bufs=1, space="PSUM"))

        xt_pool = ctx.enter_context(tc.tile_pool(name="xt", bufs=2))
        w_pool = ctx.enter_context(tc.tile_pool(name="w", bufs=4))
        wv_pool = ctx.enter_context(tc.tile_pool(name="wvp", bufs=1))
        es_pool = ctx.enter_context(tc.tile_pool(name="es", bufs=16))
        on_pool = ctx.enter_context(tc.tile_pool(name="onorm", bufs=2))
        oT_pool = ctx.enter_context(tc.tile_pool(name="oT", bufs=2))
        ysb_pool = ctx.enter_context(tc.tile_pool(name="ysb", bufs=2))
        r_pool = ctx.enter_context(tc.tile_pool(name="recip", bufs=2))

        wv_sb = wv_pool.tile([128, 8, CD], BF16, tag="wv")

        # ---------------- phase 1 helpers ----------------
        def ph1_load_xt(half, xt_sb):
            t0 = half * 1024
            for n2 in range(2):
                for kc in range(8):
                    c0 = t0 + n2 * 512
                    # split the startup-gating loads across two queues
                    if half == 1:
                        eng = nc.sync
                    else:
                        eng = nc.sync if (n2 == 0 and kc < 6) else nc.gpsimd
                    eng.dma_start(
                        out=xt_sb[:, kc, n2 * 512 : (n2 + 1) * 512],
                        in_=xT[kc * 128 : (kc + 1) * 128, c0 : c0 + 512],
                    )

        def ph1_qk_unit(half, xt_sb, wdram, dst, mm, n):
            t0 = half * 1024
            if n == 0:
                wt = w_pool.tile([128, 8, 128], BF16, tag="w")
                nc.scalar.dma_start(
                    out=wt[:],
                    in_=wdram[:, mm * 128 : (mm + 1) * 128].rearrange(
                        "(c p) m -> p c m", p=128
                    ),
                )
                ph1_qk_unit.wt = wt
            wt = ph1_qk_unit.wt
            ps = qkv_ps.tile([128, 512], F32, tag="qkvps")
            for kc in range(8):
                nc.tensor.matmul(
                    ps[:],
                    lhsT=(wt[:, kc, :]),
                    rhs=(xt_sb[:, kc, n * 512 : (n + 1) * 512]),
                    start=(kc == 0),
                    stop=(kc == 7),
                )
            col = t0 + n * 512
            nc.vector.tensor_copy(out=dst[:, mm, col : col + 512], in_=ps[:])

        def ph1_v_unit(half, xt_sb, tc8):
            tg = half * 8 + tc8
            ps = qkv_ps.tile([128, 512], F32, tag="qkvps")
            for kc in range(8):
                nc.tensor.matmul(
                    ps[:],
                    lhsT=(xt_sb[:, kc, tc8 * 128 : (tc8 + 1) * 128]),
                    rhs=(wv_sb[:, kc, :]),
                    start=(kc == 0),
                    stop=(kc == 7),
                )
            nc.vector.tensor_copy(
                out=vx[:, tg, :, 0:HD],
                in_=ps[:].rearrange("p (h d) -> p h d", h=HPC),
            )

        # ---------------- phase 1, half 0 (emitted eagerly) ----------------
        xt0 = xt_pool.tile([128, 8, 1024], BF16, tag="xt")
        ph1_load_xt(0, xt0)
        for kc in range(8):
            nc.gpsimd.dma_start(
                out=wv_sb[:, kc, :], in_=wv[kc * 128 : (kc + 1) * 128, :]
            )
        # wp/bias are not needed until the first output-projection piece
        nc.gpsimd.dma_start(
            out=wp_sb[:], in_=wp[:].rearrange("(c p) e -> p c e", p=128)
        )
        nc.gpsimd.dma_start(out=bias_sb[:], in_=bias[:].partition_broadcast(128))
        for wdram, dst in ((wq, qt), (wk, kt)):
            for mm in range(4):
                for n in range(2):
                    ph1_qk_unit(0, xt0, wdram, dst, mm, n)
        for tc8 in range(8):
            ph1_v_unit(0, xt0, tc8)

        # ---------------- phase 1, half 1 (units, interleaved later) -------
        xt1 = xt_pool.tile([128, 8, 1024], BF16, tag="xt")
        ph1_load_xt(1, xt1)
        ph1_units = deque()
        for wdram, dst in ((wq, qt), (wk, kt)):
            for mm in range(4):
                for n in range(2):
                    ph1_units.append(
                        lambda w=wdram, d=dst, m=mm, nn=n: ph1_qk_unit(
                            1, xt1, w, d, m, nn
                        )
                    )
        for tc8 in range(8):
            ph1_units.append(lambda t=tc8: ph1_v_unit(1, xt1, t))

        # ---------------- attention ----------------
        # filler queue: (pe_rows_estimate, emit_fn)
        fillers = deque()

        def pop_fillers(budget_rows):
            while fillers and budget_rows > 0:
                rows, fn = fillers.popleft()
                fn()
                budget_rows -= rows

        o_norm_tiles = {}

        def make_av_units(h, qc, es_tiles):
            """AV sweep + normalize units for (h, qc). es_tiles[g] holds kb
            (2g, 2g+1). Deferred: they pop during the NEXT head's QK/exp."""
            m, e = h // 2, h % 2
            state = {}

            def av_open():
                state["o_ps"] = o_pool.tile([128, 4, HD + 1], F32, tag="ops", name="o_ps")

            def av_qj(qj):
                o_ps = state["o_ps"]
                kb_last = 4 * qc + qj
                for kb in range(kb_last + 1):
                    g, j = kb // 2, kb % 2
                    nc.tensor.matmul(
                        out=o_ps[:, qj, :],
                        lhsT=(es_tiles[g][:, j, qj * 128 : (qj + 1) * 128]),
                        rhs=(vx[:, kb, h, :]),
                        start=(kb == 0),
                        stop=(kb == kb_last),
                    )

            def av_norm():
                o_ps = state["o_ps"]
                recip = r_pool.tile([128, 4], F32, tag="recip")
                nc.vector.reciprocal(recip[:], o_ps[:, :, HD])
                if e == 0:
                    o_norm_tiles[m] = on_pool.tile([128, 4, 128], BF16, tag="onorm", name="o_norm")
                o_norm = o_norm_tiles[m]
                for qj in range(4):
                    nc.vector.tensor_scalar_mul(
                        o_norm[:, qj, e * HD : (e + 1) * HD],
                        o_ps[:, qj, 0:HD],
                        recip[:, qj : qj + 1],
                    )
                if e == 1:
                    oT = state["oT_tile"]
                    for qj in range(4):
                        nc.sync.dma_start_transpose(
                            out=oT[:, m, qj * 128 : (qj + 1) * 128],
                            in_=o_norm[:, qj, :],
                        )

            def unit01():
                av_open()
                av_qj(0)
                av_qj(1)

            def unit23():
                av_qj(2)
                av_qj(3)

            rows01 = (4 * qc + 1 + 4 * qc + 2) * (HD + 1)
            rows23 = (4 * qc + 3 + 4 * qc + 4) * (HD + 1)
            fillers.append((rows01, unit01))
            fillers.append((rows23, unit23))
            fillers.append((100, av_norm))
            return state

        def make_y_units(qc, oT):
            def y_piece(tc4, ncol):
                row = qc * 512 + tc4 * 128
                y_ps = y_pool.tile([128, 512], F32, tag="y")
                for kc in range(4):
                    nc.tensor.matmul(
                        y_ps[:],
                        lhsT=(oT[:, kc, tc4 * 128 : (tc4 + 1) * 128]),
                        rhs=(wp_sb[:, kc, ncol * 512 : (ncol + 1) * 512]),
                        start=(kc == 0),
                        stop=(kc == 3),
                    )
                y_sb = ysb_pool.tile([128, 512], BF16, tag="ysb")
                nc.vector.tensor_add(
                    y_sb[:],
                    y_ps[:],
                    bias_sb[:, 0, ncol * 512 : (ncol + 1) * 512],
                )
                nc.sync.dma_start(
                    out=y[row : row + 128, ncol * 512 : (ncol + 1) * 512],
                    in_=y_sb[:],
                )

            for tc4 in range(4):
                for ncol in range(2):
                    fillers.append(
                        (2048, lambda t=tc4, n=ncol: y_piece(t, n))
                    )

        # main loop
        slot_idx = 0
        for qc in range(NQC):
            oT = oT_pool.tile([128, 4, 512], BF16, tag="oT")
            ngroups = 2 * qc + 2
            for m in range(4):
                for e in range(2):
                    h = 2 * m + e
                    po = e * HD
                    es_tiles = []
                    for g in range(ngroups):
                        s_ps = s_pool.tile([128, 2, 512], F32, tag="s")
                        es = es_pool.tile([128, 2, 512], BF16, tag="es")
                        es_tiles.append(es)
                        nqs = []
                        for j in range(2):
                            kb = 2 * g + j
                            r = kb * 128 - qc * 512
                            q0 = max(r, 0)
                            nq = 512 - q0
                            nqs.append((j, kb, r, q0, nq))
                            nc.tensor.matmul(
                                out=s_ps[:, j, q0:512],
                                lhsT=(kt[po : po + HD, m, kb * 128 : (kb + 1) * 128]),
                                rhs=(qt[po : po + HD, m, qc * 512 + q0 : (qc + 1) * 512]),
                                start=True,
                                stop=True,
                            )
                        if all(nq == 512 for (_, _, _, _, nq) in nqs):
                            nc.scalar.activation(out=es[:], in_=s_ps[:], func=_EXP)
                        else:
                            for j, kb, r, q0, nq in nqs:
                                nc.scalar.activation(
                                    out=es[:, j, q0:512],
                                    in_=s_ps[:, j, q0:512],
                                    func=_EXP,
                                )
                        for j, kb, r, q0, nq in nqs:
                            if r >= 0:
                                nc.gpsimd.tensor_mul(
                                    es[:, j, q0 : q0 + 128],
                                    es[:, j, q0 : q0 + 128],
                                    tri[:],
                                )
                        # PE filler while the exp chain runs on Act
                        pop_fillers(1200)
                        slot_idx += 1
                        if qc <= 1 and ph1_units and slot_idx % 2 == 0:
                            ph1_units.popleft()()
                    state = make_av_units(h, qc, es_tiles)
                    state["oT_tile"] = oT
            make_y_units(qc, oT)

        # drain any remaining phase-1 and filler work
        while ph1_units:
            ph1_units.popleft()()
        pop_fillers(10**9)


_MODULE = None


def _get_module():
    global _MODULE
    if _MODULE is None:
        _MODULE = _build_module()
    return _MODULE


def _make_in_maps(x, W_qkv, W_proj, b_proj):
    import ml_dtypes

    bf16 = ml_dtypes.bfloat16
    scale = np.float32(1.0 / np.sqrt(HD))
    bias_half = (np.asarray(b_proj, dtype=np.float32) * 0.5).reshape(1, EMB)
    in_maps = []
    for c in range(NCORES):
        b, hg = c // 2, c % 2
        cols = slice(hg * CD, (hg + 1) * CD)
        in_maps.append(
            {
                "xT": np.ascontiguousarray(
                    np.asarray(x[b], dtype=np.float32).T
                ).astype(bf16),
                "wq": (np.ascontiguousarray(W_qkv[:, 0:EMB][:, cols]) * scale).astype(
                    bf16
                ),
                "wk": np.ascontiguousarray(W_qkv[:, EMB : 2 * EMB][:, cols]).astype(
                    bf16
                ),
                "wv": np.ascontiguousarray(W_qkv[:, 2 * EMB : 3 * EMB][:, cols]).astype(
                    bf16
                ),
                "wp": np.ascontiguousarray(W_proj[cols, :]).astype(bf16),
                "bias": bias_half,
            }
        )
    return in_maps


def kernel(x, W_qkv, W_proj, b_proj, _trace=False, _trace_kwargs=None):
    x = np.asarray(x, dtype=np.float32)
    W_qkv = np.asarray(W_qkv, dtype=np.float32)
    W_proj = np.asarray(W_proj, dtype=np.float32)
    b_proj = np.asarray(b_proj, dtype=np.float32)

    nc = _get_module()
    in_maps = _make_in_maps(x, W_qkv, W_proj, b_proj)
    res = run_bass_kernel_spmd(
        nc, in_maps, list(range(NCORES)), trace=_trace, **(_trace_kwargs or {})
    )
    out = np.empty((B, S, EMB), dtype=np.float32)
    for b in range(B):
        out[b] = res.results[2 * b]["y"].astype(np.float32) + res.results[
            2 * b + 1
        ]["y"].astype(np.float32)
    if _trace:
        return out, res
    return out
